# revision 32
# baseline (speedup 1.0000x reference)
"""Trainium2 Bass kernel for nn_MultiHeadAttention_75754633167392.

Multi-head attention with a dynamic per-query conv1d over keys
(per-head kernel widths KWS = [1,1,1,1,3,3,5,7], zero-padded to 7 taps).

Sharding: pure data-parallel over batch — B == n_cores == 8, one batch
element per NeuronCore, no collectives.

Per-core algorithm (fp32 data, float32r matmuls):
  - host pre-transposes q/k/v to (D, L) so contraction over D feeds the
    PE naturally; all weights are host-packed into lhsT layouts.
  - attention logits S[i,j] = bias_q[i] + bias_b
        + sum_t (q_s @ WkerT_t + bker_t)[i,:] . k_pad[j+t,:]
    are evaluated as matmuls over a stacked contraction axis: pairs of
    adjacent taps (t, t+1) are stacked into 128-row operands, using a
    key buffer kTD whose lower 64 partitions hold k_sT and whose upper
    64 partitions hold k_sT shifted by one key position.  The per-query
    bias rides along as one extra contraction row (lhs row = bias_tot[i],
    rhs row = 1).  Only nonzero taps are computed.
  - S is computed in BOTH orientations on the PE (j-major first for the
    attn @ v contraction, then i-major for the attn output); operands are
    identical, only stationary/moving roles swap.
  - the j-major phase also yields the softmax denominators (ones column
    appended to v); the i-major exp then applies bias = -ln(sums[i]) so
    the attn output comes out of the Activation engine already
    normalized (softmax skips max-subtraction: logit rowmax <= ~15).
  - per-head 64-row operands for even/odd heads live in the lower/upper
    64 partitions of shared tiles; matmuls address them via row/col
    tile_position offsets.
"""

import numpy as np

import concourse.bass as bass
import concourse.bacc as bacc
import concourse.mybir as mybir
import concourse.tile as tile
from concourse.bass_utils import run_bass_kernel_spmd

F32 = mybir.dt.float32
F32R = mybir.dt.float32r
LAST_RES = None
import os
INTERLEAVE = os.environ.get("KERN_INTERLEAVE", "0") == "1"
PIPE2 = os.environ.get("KERN_PIPE2", "1") == "1"

B, L, D, H, DK, KW, PAD = 8, 1024, 512, 8, 64, 7, 3
KWS = [1, 1, 1, 1, 3, 3, 5, 7]
TEMPER = float(DK) ** 0.5
NCORES = 8
NIB = L // 128            # 8 row blocks of 128
NIC = L // 512            # 2 column chunks of 512
NKB = D // 128            # 4 contraction blocks of 128
KTDW = L + 8              # key buffer width (3+1024+5 incl. shift slack)


def _head_taps(h):
    kw = KWS[h]
    off = (KW - kw) // 2
    return list(range(off, off + kw))


def _head_blocks(h):
    """Split the head's nonzero taps into adjacent pairs + one tail tap."""
    taps = _head_taps(h)
    pairs = []
    while len(taps) > 1:
        pairs.append((taps[0], taps[1]))
        taps = taps[2:]
    return pairs, taps[0]


# Wker matmul tiles: per (head, pair) a (128, 128) lhsT with tap A weights in
# columns 0-63 and tap B in 64-127, rows 64*(h%2)..+64 (zeros elsewhere so the
# full-128 contraction with the shared q_sT tile nulls the sibling head).
# Per head one (128, 65) tail tile: tap columns 0-63 plus Wqb bias column 64.
_PAIR_IDX = {}
for _h in range(H):
    _pairs, _ = _head_blocks(_h)
    for _pi in range(len(_pairs)):
        _PAIR_IDX[(_h, _pi)] = len(_PAIR_IDX)
N_PAIRS = len(_PAIR_IDX)  # 7

_BKER_IDX = {}
for _h in range(H):
    _pairs, _tail = _head_blocks(_h)
    for _pi, _ in enumerate(_pairs):
        _BKER_IDX[(_h, _pi)] = len(_BKER_IDX)
    _BKER_IDX[(_h, "tail")] = len(_BKER_IDX)
N_BKER = len(_BKER_IDX)   # 15

# cols_pack column layout: bq_pair (4) | Wqb_pair (4) | bk_dup (8) | bker (15)
_COL_BQ = 0
_COL_WQB = 4
_COL_BK = 8
_COL_BKER = 16
N_COLS = 16 + N_BKER


def _mmr(nc, out, lhsT, rhs, **kw):
    nc.tensor.matmul(out, lhsT, rhs, **kw)


def build_program(cb):
    """Build the single-core Bass program. cb[h] = bqb[h] + bias_b[h]."""
    nc = bacc.Bacc(None, target_bir_lowering=False)

    qT_d = nc.dram_tensor("qTp", [128, NKB * L], F32R, kind="ExternalInput")
    kT_d = nc.dram_tensor("kTp", [128, NKB * L], F32R, kind="ExternalInput")
    vT_d = nc.dram_tensor("vTp", [128, NKB * L], F32R, kind="ExternalInput")
    WqT_d = nc.dram_tensor("WqTp", [128, NKB * D], F32R, kind="ExternalInput")
    WvT_d = nc.dram_tensor("WvTp", [128, NKB * D], F32R, kind="ExternalInput")
    WpT_d = nc.dram_tensor("WpTp", [128, NKB * D], F32R, kind="ExternalInput")
    Wkd_d = nc.dram_tensor("Wk_dup", [H, 128, D], F32R, kind="ExternalInput")
    Wkp_d = nc.dram_tensor("wker_pairs", [N_PAIRS, 128, 128], F32R, kind="ExternalInput")
    Wkt_d = nc.dram_tensor("wker_tails", [H, 128, 65], F32R, kind="ExternalInput")
    cols_d = nc.dram_tensor("cols_pack", [128, N_COLS], F32, kind="ExternalInput")
    bv_d = nc.dram_tensor("bv_bc", [128, D], F32, kind="ExternalInput")
    bp_d = nc.dram_tensor("bproj_bc", [128, D], F32, kind="ExternalInput")

    out_d = nc.dram_tensor("out", [L, D], F32, kind="ExternalOutput")
    attn_d = nc.dram_tensor("attn", [H, L, L], F32, kind="ExternalOutput")
    oha_d = nc.dram_tensor("oha", [L, L], F32, kind="ExternalOutput")

    inv_temper = 1.0 / TEMPER

    with tile.TileContext(nc) as tc:
        with (
            tc.tile_pool(name="persist", bufs=1) as pp,
            tc.tile_pool(name="ktd", bufs=2) as ktd_pool,
            tc.tile_pool(name="wkd", bufs=2) as wkd_pool,
            tc.tile_pool(name="stack", bufs=2) as stack_pool,
            tc.tile_pool(name="work", bufs=2) as work,
            tc.tile_pool(name="rows", bufs=1) as rows_pool,
            tc.tile_pool(name="psS", bufs=int(os.environ.get("PSS_BUFS", "4")), space="PSUM") as psS,
            tc.tile_pool(name="psctx", bufs=int(os.environ.get("PSCTX_BUFS", "1")), space="PSUM") as psctx,
            tc.tile_pool(name="pssmall", bufs=int(os.environ.get("PSSM_BUFS", "2")), space="PSUM") as pssm,
        ):
            # ---------- persistent operands ----------
            kT = pp.tile([128, NKB * L], F32R, name="kT")
            nc.sync.dma_start(kT[:], kT_d[:])
            Wkp = {}
            for (hh, pi), idx in _PAIR_IDX.items():
                w = pp.tile([128, 128], F32R, name=f"Wkp{idx}")
                nc.sync.dma_start(w[:], Wkp_d[idx, :, :])
                Wkp[(hh, pi)] = w
            Wkt = []
            for hh in range(H):
                w = pp.tile([128, 65], F32R, name=f"Wkt{hh}")
                nc.sync.dma_start(w[:], Wkt_d[hh, :, :])
                Wkt.append(w)
            cols = pp.tile([128, N_COLS], F32, name="cols")
            nc.sync.dma_start(cols[:], cols_d[:])
            bv_bc = pp.tile([128, D], F32, name="bv_bc")
            nc.sync.dma_start(bv_bc[:], bv_d[:])
            bp_bc = pp.tile([128, D], F32, name="bp_bc")
            nc.sync.dma_start(bp_bc[:], bp_d[:])
            WpT = pp.tile([128, NKB * D], F32R, name="WpT")
            nc.sync.dma_start(WpT[:], WpT_d[:])
            ones_col = pp.tile([1, 64], F32, name="ones_col")
            nc.vector.memset(ones_col[:], 1.0)
            neg1 = pp.tile([1, 1], F32, name="neg1")
            nc.vector.memset(neg1[:], -1.0)

            v_aug = [pp.tile([128, 65 * H], F32R, name=f"vaug{j}") for j in range(NIB)]
            q_sT = [pp.tile([128, L], F32R, name=f"qsT{p}") for p in range(H // 2)]
            ctxT = [pp.tile([128, L], F32R, name=f"ctxT{i}") for i in range(NKB)]

            # ---------- prep phase (transient tiles): q_sT, then v_aug ----------
            with tc.tile_pool(name="prepq", bufs=1) as prep:
                qT = prep.tile([128, NKB * L], F32R, name="qT")
                WqT = prep.tile([128, NKB * D], F32R, name="WqT")
                nc.sync.dma_start(qT[:], qT_d[:])
                nc.sync.dma_start(WqT[:], WqT_d[:])

                # q projection: head pair p -> q_sT[p] (even head rows 0-63)
                for p in range(H // 2):
                    for ic in range(NIC):
                        pq = pssm.tile([128, 512], F32, name="ps_small",
                                       tag="ps_small")
                        for kb in range(NKB):
                            _mmr(nc, pq[:],
                                 WqT[:, D * kb + 128 * p:D * kb + 128 * (p + 1)],
                                 qT[:, L * kb + 512 * ic:L * kb + 512 * (ic + 1)],
                                 start=(kb == 0), stop=(kb == NKB - 1))
                        nc.vector.tensor_scalar_add(
                            q_sT[p][:, 512 * ic:512 * (ic + 1)], pq[:],
                            cols[:, _COL_BQ + p:_COL_BQ + p + 1])

            with tc.tile_pool(name="prepv", bufs=1) as prep:
                vT = prep.tile([128, NKB * L], F32R, name="vT")
                WvT = prep.tile([128, NKB * D], F32R, name="WvT")
                nc.sync.dma_start(vT[:], vT_d[:])
                nc.sync.dma_start(WvT[:], WvT_d[:])

                # v projection -> v_aug (interleaved [64 v | 1 ones] per head)
                for j in range(NIB):
                    pv = pssm.tile([128, D], F32, name="ps_small", tag="ps_small")
                    for kb in range(NKB):
                        _mmr(nc, pv[:],
                             vT[:, L * kb + 128 * j:L * kb + 128 * (j + 1)],
                             WvT[:, D * kb:D * (kb + 1)],
                             start=(kb == 0), stop=(kb == NKB - 1))
                    for h in range(H):
                        nc.vector.tensor_tensor(
                            out=v_aug[j][:, 65 * h:65 * h + 64],
                            in0=pv[:, 64 * h:64 * (h + 1)],
                            in1=bv_bc[:, 64 * h:64 * (h + 1)],
                            op=mybir.AluOpType.add)
                    ones_v = v_aug[j].rearrange("p (h c) -> p h c", c=65)[:, :, 64:65]
                    nc.vector.memset(ones_v.bitcast(F32), 1.0)

            # ---------- head loop (software-pipelined) ----------
            # Head h's i-major (attn-output) blocks are emitted interleaved
            # with head h+1's j-major blocks so PSUM slots and engines
            # alternate between the two phases with no serial join.

            def emit_natural_block(st8, ib):
                hh, stacks_, negln_ = st8[0], st8[1], st8[2]
                if ib % 2 == 0:
                    st8[3] = work.tile([128, 2 * L], F32, name="attn_t",
                                       tag="attn_t", bufs=2)
                attn_t = st8[3]
                for jc in range(NIC):
                    pS = psS.tile([128, 512], F32, name="psS", tag="psS")
                    for bi, (st, kbuf, tA) in enumerate(stacks_):
                        _mmr(nc, pS[:],
                             st[:, 128 * ib:128 * (ib + 1)],
                             kbuf[:, 512 * jc + tA:512 * jc + tA + 512],
                             start=(bi == 0), stop=(bi == len(stacks_) - 1))
                    half = L * (ib % 2) + 512 * jc
                    nc.scalar.activation(attn_t[:, half:half + 512], pS[:],
                                         mybir.ActivationFunctionType.Exp,
                                         scale=inv_temper,
                                         bias=negln_[:, ib:ib + 1])
                if ib % 2 == 1:
                    src = attn_t.rearrange("p (c f) -> p c f", c=2)
                    qtr = ib // 2
                    dst = attn_d[hh, 256 * qtr:256 * (qtr + 1), :].rearrange(
                        "(c p) f -> p c f", p=128)
                    nc.sync.dma_start(dst, src)
                    if hh == 0:
                        dsto = oha_d[256 * qtr:256 * (qtr + 1), :].rearrange(
                            "(c p) f -> p c f", p=128)
                        nc.sync.dma_start(dsto, src)

            pending = None
            for h in range(H):
                p, r = h // 2, 64 * (h % 2)
                pairs, tail = _head_blocks(h)
                qs = q_sT[p]

                # --- k projection (duplicated rows) into shifted key buffers ---
                # kTD rows 0-63:  k_sT[h][c, u] at columns u = j + 3   (k_pad index)
                # kTD rows 64-127: same shifted by one key position
                # kTD2: rows 0-63 as kTD, row 64 = ones (bias lane), rows 65+ = 0
                wkd = wkd_pool.tile([128, D], F32R, name="wkd", tag="wkd")
                nc.sync.dma_start(wkd[:], Wkd_d[h, :, :])
                ktd2 = ktd_pool.tile([128, KTDW], F32R, name="ktd2", tag="ktd2")
                nc.gpsimd.memset(ktd2[64:128, :].bitcast(F32), 0.0)
                nc.gpsimd.memset(ktd2[64:65, :].bitcast(F32), 1.0)
                nc.vector.memset(ktd2[0:64, 0:3].bitcast(F32), 0.0)
                nc.vector.memset(ktd2[0:64, 3 + L:KTDW].bitcast(F32), 0.0)
                ktd = None
                if pairs:
                    ktd = ktd_pool.tile([128, KTDW], F32R, name="ktd", tag="ktd")
                    nc.vector.memset(ktd[0:64, 0:3].bitcast(F32), 0.0)
                    nc.vector.memset(ktd[0:64, 3 + L:KTDW].bitcast(F32), 0.0)
                    nc.vector.memset(ktd[64:128, 0:2].bitcast(F32), 0.0)
                    nc.vector.memset(ktd[64:128, 2 + L:KTDW].bitcast(F32), 0.0)
                bk_col = cols[:, _COL_BK + h:_COL_BK + h + 1]
                for ic in range(NIC):
                    pk = pssm.tile([128, 512], F32, name="ps_small", tag="ps_small")
                    for kb in range(NKB):
                        _mmr(nc, pk[:], wkd[:, 128 * kb:128 * (kb + 1)],
                             kT[:, L * kb + 512 * ic:L * kb + 512 * (ic + 1)],
                             start=(kb == 0), stop=(kb == NKB - 1))
                    nc.vector.tensor_scalar_add(
                        ktd2[0:64, 3 + 512 * ic:3 + 512 * ic + 512],
                        pk[0:64, :], bk_col[0:64, :])
                    if pairs:
                        nc.vector.tensor_scalar_add(
                            ktd[0:64, 3 + 512 * ic:3 + 512 * ic + 512],
                            pk[0:64, :], bk_col[0:64, :])
                        nc.vector.tensor_scalar_add(
                            ktd[64:128, 2 + 512 * ic:2 + 512 * ic + 512],
                            pk[64:128, :], bk_col[64:128, :])

                # --- Kt stacks over the stacked contraction axis ---
                # pair pi: rows 0-63 = q_s @ WkerT_tA + bker_tA; rows 64-127 = tap tB
                # tail:    rows 0-63 = tap t_tail; row 64 = bias_tot[i]; rows 65+ = 0
                stacks = []  # (sbuf_tile, key_buffer, top_tap)
                for pi, (tA, tB) in enumerate(pairs):
                    st = stack_pool.tile([128, L], F32R, name=f"stack{pi}",
                                         tag=f"stack{pi}")
                    bcol = cols[:, _COL_BKER + _BKER_IDX[(h, pi)]:
                                _COL_BKER + _BKER_IDX[(h, pi)] + 1]
                    for ic in range(NIC):
                        pkt = pssm.tile([128, 512], F32, name="ps_small",
                                        tag="ps_small")
                        _mmr(nc, pkt[:], Wkp[(h, pi)][:],
                             qs[:, 512 * ic:512 * (ic + 1)],
                             start=True, stop=True)
                        nc.vector.tensor_scalar_add(
                            st[:, 512 * ic:512 * (ic + 1)], pkt[:], bcol)
                    stacks.append((st, ktd, tA))

                st_tail = stack_pool.tile([128, L], F32R, name="stack_tail",
                                          tag="stack_tail")
                bcol_t = cols[:, _COL_BKER + _BKER_IDX[(h, "tail")]:
                              _COL_BKER + _BKER_IDX[(h, "tail")] + 1]
                nc.gpsimd.memset(st_tail[64:128, :].bitcast(F32), 0.0)
                for ic in range(NIC):
                    pkt = pssm.tile([128, 512], F32, name="ps_small", tag="ps_small")
                    _mmr(nc, pkt[0:65, :], Wkt[h][:],
                         qs[:, 512 * ic:512 * (ic + 1)],
                         start=True, stop=True)
                    nc.vector.tensor_scalar_add(
                        st_tail[0:65, 512 * ic:512 * (ic + 1)], pkt[0:65, :],
                        bcol_t[0:65, :])
                stacks.append((st_tail, ktd2, tail))

                if PIPE2 and pending is not None:
                    for ib2 in range(NIB):
                        emit_natural_block(pending, ib2)
                    pending = None

                # --- j-major: S_T[j, i] -> exp -> ctx; interleave prev natural ---
                pctx = psctx.tile([65, L], F32, name="psctx", tag="psctx")
                for jb in range(NIB):
                    expT = work.tile([128, L], F32R, name="expT", tag="expT")
                    for ic in range(NIC):
                        pT = psS.tile([128, 512], F32, name="psS", tag="psS")
                        for bi, (st, kbuf, tA) in enumerate(stacks):
                            _mmr(nc, pT[:],
                                 kbuf[:, 128 * jb + tA:128 * jb + tA + 128],
                                 st[:, 512 * ic:512 * (ic + 1)],
                                 start=(bi == 0), stop=(bi == len(stacks) - 1))
                        nc.scalar.activation(expT[:, 512 * ic:512 * (ic + 1)],
                                             pT[:],
                                             mybir.ActivationFunctionType.Exp,
                                             scale=inv_temper)
                        _mmr(nc, pctx[:, 512 * ic:512 * (ic + 1)],
                             v_aug[jb][:, 65 * h:65 * (h + 1)],
                             expT[:, 512 * ic:512 * (ic + 1)],
                             start=(jb == 0), stop=(jb == NIB - 1))
                    if INTERLEAVE and pending is not None:
                        emit_natural_block(pending, jb)
                if INTERLEAVE:
                    pending = None

                # --- softmax denominators: sums row -> -ln cols + recip bcast ---
                sums_row = rows_pool.tile([1, L], F32, name="sums_row", tag="sums_row")
                nc.vector.tensor_copy(sums_row[:], pctx[64:65, :])
                ln_row = rows_pool.tile([1, L], F32, name="ln_row", tag="ln_row")
                nc.scalar.activation(ln_row[:], sums_row[:],
                                     mybir.ActivationFunctionType.Ln)
                negln = work.tile([128, NIB], F32, name="negln", tag="negln")
                for ib in range(NIB):
                    pcol = pssm.tile([128, 512], F32, name="ps_small",
                                     tag="ps_small")
                    nc.tensor.matmul(pcol[:, 0:1],
                                     ln_row[:, 128 * ib:128 * (ib + 1)],
                                     neg1[:], start=True, stop=True)
                    nc.vector.tensor_copy(negln[:, ib:ib + 1], pcol[:, 0:1])

                rcp_row = rows_pool.tile([1, L], F32, name="rcp_row", tag="rcp_row")
                nc.vector.reciprocal(rcp_row[:], sums_row[:])
                bc_sb = work.tile([64, L], F32, name="bc_sb", tag="bc_sb")
                for ic in range(NIC):
                    pbc = pssm.tile([128, 512], F32, name="ps_small", tag="ps_small")
                    nc.tensor.matmul(pbc[0:64, :], ones_col[:],
                                     rcp_row[:, 512 * ic:512 * (ic + 1)],
                                     start=True, stop=True)
                    nc.vector.tensor_copy(bc_sb[:, 512 * ic:512 * (ic + 1)],
                                          pbc[0:64, :])
                nc.vector.tensor_tensor(
                    out=ctxT[p][r:r + 64, :],
                    in0=pctx[0:64, :], in1=bc_sb[:],
                    op=mybir.AluOpType.mult)

                if INTERLEAVE or PIPE2:
                    pending = [h, stacks, negln, None]
                else:
                    own = [h, stacks, negln, None]
                    for ib in range(NIB):
                        emit_natural_block(own, ib)

            if INTERLEAVE or PIPE2:
                for ib in range(NIB):
                    emit_natural_block(pending, ib)
                pending = None

            # ---------- output projection ----------
            for ib in range(NIB):
                po = pssm.tile([128, 512], F32, name="ps_small", tag="ps_small")
                for kb in range(NKB):
                    _mmr(nc, po[:], ctxT[kb][:, 128 * ib:128 * (ib + 1)],
                         WpT[:, D * kb:D * (kb + 1)],
                         start=(kb == 0), stop=(kb == NKB - 1))
                out_t = work.tile([128, D], F32, name="out_t", tag="out_t")
                nc.vector.tensor_tensor(out=out_t[:], in0=po[:], in1=bp_bc[:],
                                        op=mybir.AluOpType.add)
                nc.sync.dma_start(out_d[128 * ib:128 * (ib + 1), :], out_t[:])

    nc.compile()
    return nc


def _pack_kblocks(x, inner):
    """(D, X) -> (128, NKB * X) with block kb at columns [kb*X, (kb+1)*X)."""
    del inner
    Dd, X = x.shape
    out = np.empty((128, NKB * X), np.float32)
    for kb in range(NKB):
        out[:, X * kb:X * (kb + 1)] = x[128 * kb:128 * (kb + 1), :]
    return out


def _prep_weights(Wq, bq, Wk, bk, Wv, bv, Wker, bker, Wqb, bqb, bias_b,
                  Wproj, bproj):
    w = {}
    w["WqTp"] = _pack_kblocks(np.ascontiguousarray(Wq.T), D)
    WkT = Wk.T
    wkd = np.zeros((H, 128, D), np.float32)
    for h in range(H):
        blk = WkT[:, 64 * h:64 * (h + 1)]           # (D, 64): [kd, c]
        dup = np.concatenate([blk, blk], axis=1)    # (D, 128): [kd, m]
        for kb in range(NKB):
            wkd[h, :, 128 * kb:128 * (kb + 1)] = dup[128 * kb:128 * (kb + 1), :]
    w["Wk_dup"] = wkd
    w["WvTp"] = _pack_kblocks(np.ascontiguousarray(Wv.T), D)
    w["WpTp"] = _pack_kblocks(np.ascontiguousarray(Wproj.T), D)
    wkp = np.zeros((N_PAIRS, 128, 128), np.float32)
    wkt = np.zeros((H, 128, 65), np.float32)
    for h in range(H):
        rr = 64 * (h % 2)
        pairs, tail = _head_blocks(h)
        for pi, (tA, tB) in enumerate(pairs):
            idx = _PAIR_IDX[(h, pi)]
            wkp[idx, rr:rr + 64, 0:64] = Wker[h, :, tA, :].T
            wkp[idx, rr:rr + 64, 64:128] = Wker[h, :, tB, :].T
        wkt[h, rr:rr + 64, 0:64] = Wker[h, :, tail, :].T
        wkt[h, rr:rr + 64, 64] = Wqb[h]
    w["wker_pairs"] = wkp
    w["wker_tails"] = wkt
    cols = np.zeros((128, N_COLS), np.float32)
    for h in range(H):
        p, r = h // 2, 64 * (h % 2)
        cols[r:r + 64, _COL_BQ + p] = bq[64 * h:64 * (h + 1)]
        cols[r:r + 64, _COL_WQB + p] = Wqb[h]
        cols[:, _COL_BK + h] = np.concatenate([bk[64 * h:64 * (h + 1)]] * 2)
        pairs, tail = _head_blocks(h)
        for pi, (tA, tB) in enumerate(pairs):
            cols[0:64, _COL_BKER + _BKER_IDX[(h, pi)]] = bker[h, :, tA]
            cols[64:128, _COL_BKER + _BKER_IDX[(h, pi)]] = bker[h, :, tB]
        cols[0:64, _COL_BKER + _BKER_IDX[(h, "tail")]] = bker[h, :, tail]
        cols[64, _COL_BKER + _BKER_IDX[(h, "tail")]] = bqb[h] + bias_b[h]
    w["cols_pack"] = cols
    w["bv_bc"] = np.ascontiguousarray(np.broadcast_to(bv, (128, D)))
    w["bproj_bc"] = np.ascontiguousarray(np.broadcast_to(bproj, (128, D)))
    cb = [float(bqb[h] + bias_b[h]) for h in range(H)]
    return w, cb


def kernel(q, k, v, attn_mask, Wq, bq, Wk, bk, Wv, bv,
           Wker, bker, Wqb, bqb, bias_b, Wproj, bproj):
    del attn_mask  # all-False per the input spec; where(mask, -inf, .) is a no-op
    arrs = {n: np.asarray(a, np.float32) for n, a in dict(
        q=q, k=k, v=v, Wq=Wq, bq=bq, Wk=Wk, bk=bk, Wv=Wv, bv=bv, Wker=Wker,
        bker=bker, Wqb=Wqb, bqb=bqb, bias_b=bias_b, Wproj=Wproj,
        bproj=bproj).items()}

    w, cb = _prep_weights(
        arrs["Wq"], arrs["bq"], arrs["Wk"], arrs["bk"], arrs["Wv"], arrs["bv"],
        arrs["Wker"], arrs["bker"], arrs["Wqb"], arrs["bqb"], arrs["bias_b"],
        arrs["Wproj"], arrs["bproj"])

    nc = build_program(cb)

    in_maps = []
    for b in range(NCORES):
        m = dict(w)
        m["qTp"] = _pack_kblocks(arrs["q"][b].T, L)
        m["kTp"] = _pack_kblocks(arrs["k"][b].T, L)
        m["vTp"] = _pack_kblocks(arrs["v"][b].T, L)
        in_maps.append(m)

    global LAST_RES
    res = run_bass_kernel_spmd(nc, in_maps, list(range(NCORES)))
    LAST_RES = res
    results = res.results

    out = np.stack([results[b]["out"] for b in range(NCORES)])
    attn = np.stack([results[b]["attn"] for b in range(NCORES)])
    oha = np.stack([results[b]["oha"] for b in range(NCORES)])
    return out, attn, oha


# revision 34
# speedup vs baseline: 1.0076x; 1.0076x over previous
"""Trainium2 Bass kernel for nn_MultiHeadAttention_75754633167392.

Multi-head attention with a dynamic per-query conv1d over keys
(per-head kernel widths KWS = [1,1,1,1,3,3,5,7], zero-padded to 7 taps).

Sharding: pure data-parallel over batch — B == n_cores == 8, one batch
element per NeuronCore, no collectives.

Per-core algorithm (fp32 data, float32r matmuls):
  - host pre-transposes q/k/v to (D, L) so contraction over D feeds the
    PE naturally; all weights are host-packed into lhsT layouts.
  - attention logits S[i,j] = bias_q[i] + bias_b
        + sum_t (q_s @ WkerT_t + bker_t)[i,:] . k_pad[j+t,:]
    are evaluated as matmuls over a stacked contraction axis: pairs of
    adjacent taps (t, t+1) are stacked into 128-row operands, using a
    key buffer kTD whose lower 64 partitions hold k_sT and whose upper
    64 partitions hold k_sT shifted by one key position.  The per-query
    bias rides along as one extra contraction row (lhs row = bias_tot[i],
    rhs row = 1).  Only nonzero taps are computed.
  - S is computed in BOTH orientations on the PE (j-major first for the
    attn @ v contraction, then i-major for the attn output); operands are
    identical, only stationary/moving roles swap.
  - the j-major phase also yields the softmax denominators (ones column
    appended to v); the i-major exp then applies bias = -ln(sums[i]) so
    the attn output comes out of the Activation engine already
    normalized (softmax skips max-subtraction: logit rowmax <= ~15).
  - per-head 64-row operands for even/odd heads live in the lower/upper
    64 partitions of shared tiles; matmuls address them via row/col
    tile_position offsets.
"""

import numpy as np

import concourse.bass as bass
import concourse.bacc as bacc
import concourse.mybir as mybir
import concourse.tile as tile
from concourse.bass_utils import run_bass_kernel_spmd

F32 = mybir.dt.float32
F32R = mybir.dt.float32r
LAST_RES = None
import os
INTERLEAVE = os.environ.get("KERN_INTERLEAVE", "0") == "1"
PIPE2 = os.environ.get("KERN_PIPE2", "1") == "1"

B, L, D, H, DK, KW, PAD = 8, 1024, 512, 8, 64, 7, 3
KWS = [1, 1, 1, 1, 3, 3, 5, 7]
TEMPER = float(DK) ** 0.5
NCORES = 8
NIB = L // 128            # 8 row blocks of 128
NIC = L // 512            # 2 column chunks of 512
NKB = D // 128            # 4 contraction blocks of 128
KTDW = L + 8              # key buffer width (3+1024+5 incl. shift slack)


def _head_taps(h):
    kw = KWS[h]
    off = (KW - kw) // 2
    return list(range(off, off + kw))


def _head_blocks(h):
    """Split the head's nonzero taps into adjacent pairs + one tail tap."""
    taps = _head_taps(h)
    pairs = []
    while len(taps) > 1:
        pairs.append((taps[0], taps[1]))
        taps = taps[2:]
    return pairs, taps[0]


# Wker matmul tiles: per (head, pair) a (128, 128) lhsT with tap A weights in
# columns 0-63 and tap B in 64-127, rows 64*(h%2)..+64 (zeros elsewhere so the
# full-128 contraction with the shared q_sT tile nulls the sibling head).
# Per head one (128, 65) tail tile: tap columns 0-63 plus Wqb bias column 64.
_PAIR_IDX = {}
for _h in range(H):
    _pairs, _ = _head_blocks(_h)
    for _pi in range(len(_pairs)):
        _PAIR_IDX[(_h, _pi)] = len(_PAIR_IDX)
N_PAIRS = len(_PAIR_IDX)  # 7

_BKER_IDX = {}
for _h in range(H):
    _pairs, _tail = _head_blocks(_h)
    for _pi, _ in enumerate(_pairs):
        _BKER_IDX[(_h, _pi)] = len(_BKER_IDX)
    _BKER_IDX[(_h, "tail")] = len(_BKER_IDX)
N_BKER = len(_BKER_IDX)   # 15

# cols_pack column layout: bq_pair (4) | Wqb_pair (4) | bk_dup (8) | bker (15)
_COL_BQ = 0
_COL_WQB = 4
_COL_BK = 8
_COL_BKER = 16
N_COLS = 16 + N_BKER


def _mmr(nc, out, lhsT, rhs, **kw):
    nc.tensor.matmul(out, lhsT, rhs, **kw)


def build_program(cb):
    """Build the single-core Bass program. cb[h] = bqb[h] + bias_b[h]."""
    nc = bacc.Bacc(None, target_bir_lowering=False)

    qT_d = nc.dram_tensor("qTp", [128, NKB * L], F32R, kind="ExternalInput")
    kT_d = nc.dram_tensor("kTp", [128, NKB * L], F32R, kind="ExternalInput")
    vT_d = nc.dram_tensor("vTp", [128, NKB * L], F32R, kind="ExternalInput")
    WqT_d = nc.dram_tensor("WqTp", [128, NKB * D], F32R, kind="ExternalInput")
    WvT_d = nc.dram_tensor("WvTp", [128, NKB * D], F32R, kind="ExternalInput")
    WpT_d = nc.dram_tensor("WpTp", [128, NKB * D], F32R, kind="ExternalInput")
    Wkd_d = nc.dram_tensor("Wk_dup", [H, 128, D], F32R, kind="ExternalInput")
    Wkp_d = nc.dram_tensor("wker_pairs", [N_PAIRS, 128, 128], F32R, kind="ExternalInput")
    Wkt_d = nc.dram_tensor("wker_tails", [H, 128, 65], F32R, kind="ExternalInput")
    cols_d = nc.dram_tensor("cols_pack", [128, N_COLS], F32, kind="ExternalInput")
    bv_d = nc.dram_tensor("bv_bc", [128, D], F32, kind="ExternalInput")
    bp_d = nc.dram_tensor("bproj_bc", [128, D], F32, kind="ExternalInput")

    out_d = nc.dram_tensor("out", [L, D], F32, kind="ExternalOutput")
    attn_d = nc.dram_tensor("attn", [H, L, L], F32, kind="ExternalOutput")
    oha_d = nc.dram_tensor("oha", [L, L], F32, kind="ExternalOutput")

    inv_temper = 1.0 / TEMPER

    with tile.TileContext(nc) as tc:
        with (
            tc.tile_pool(name="persist", bufs=1) as pp,
            tc.tile_pool(name="ktd", bufs=2) as ktd_pool,
            tc.tile_pool(name="wkd", bufs=2) as wkd_pool,
            tc.tile_pool(name="stack", bufs=2) as stack_pool,
            tc.tile_pool(name="work", bufs=2) as work,
            tc.tile_pool(name="rows", bufs=1) as rows_pool,
            tc.tile_pool(name="psS", bufs=int(os.environ.get("PSS_BUFS", "4")), space="PSUM") as psS,
            tc.tile_pool(name="psctx", bufs=int(os.environ.get("PSCTX_BUFS", "1")), space="PSUM") as psctx,
            tc.tile_pool(name="pssmall", bufs=int(os.environ.get("PSSM_BUFS", "2")), space="PSUM") as pssm,
        ):
            # ---------- persistent operands ----------
            kT = pp.tile([128, NKB * L], F32R, name="kT")
            nc.sync.dma_start(kT[:], kT_d[:])
            Wkp = {}
            for (hh, pi), idx in _PAIR_IDX.items():
                w = pp.tile([128, 128], F32R, name=f"Wkp{idx}")
                nc.sync.dma_start(w[:], Wkp_d[idx, :, :])
                Wkp[(hh, pi)] = w
            Wkt = []
            for hh in range(H):
                w = pp.tile([128, 65], F32R, name=f"Wkt{hh}")
                nc.sync.dma_start(w[:], Wkt_d[hh, :, :])
                Wkt.append(w)
            cols = pp.tile([128, N_COLS], F32, name="cols")
            nc.sync.dma_start(cols[:], cols_d[:])
            bv_bc = pp.tile([128, D], F32, name="bv_bc")
            nc.sync.dma_start(bv_bc[:], bv_d[:])
            bp_bc = pp.tile([128, D], F32, name="bp_bc")
            nc.sync.dma_start(bp_bc[:], bp_d[:])
            WpT = pp.tile([128, NKB * D], F32R, name="WpT")
            nc.sync.dma_start(WpT[:], WpT_d[:])
            ones_col = pp.tile([1, 64], F32, name="ones_col")
            nc.vector.memset(ones_col[:], 1.0)
            neg1 = pp.tile([1, 1], F32, name="neg1")
            nc.vector.memset(neg1[:], -1.0)

            v_aug = [pp.tile([128, 65 * H], F32R, name=f"vaug{j}") for j in range(NIB)]
            q_sT = [pp.tile([128, L], F32R, name=f"qsT{p}") for p in range(H // 2)]
            ctxT = [pp.tile([128, L], F32R, name=f"ctxT{i}") for i in range(NKB)]

            # ---------- k-projection / key-buffer builder ----------
            def emit_kproj(h):
                pairs_, _tail = _head_blocks(h)
                wkd = wkd_pool.tile([128, D], F32R, name="wkd", tag="wkd")
                nc.sync.dma_start(wkd[:], Wkd_d[h, :, :])
                ktd2 = ktd_pool.tile([128, KTDW], F32R, name="ktd2", tag="ktd2")
                nc.gpsimd.memset(ktd2[64:128, :].bitcast(F32), 0.0)
                nc.gpsimd.memset(ktd2[64:65, :].bitcast(F32), 1.0)
                nc.vector.memset(ktd2[0:64, 0:3].bitcast(F32), 0.0)
                nc.vector.memset(ktd2[0:64, 3 + L:KTDW].bitcast(F32), 0.0)
                ktd = None
                if pairs_:
                    ktd = ktd_pool.tile([128, KTDW], F32R, name="ktd", tag="ktd")
                    nc.vector.memset(ktd[0:64, 0:3].bitcast(F32), 0.0)
                    nc.vector.memset(ktd[0:64, 3 + L:KTDW].bitcast(F32), 0.0)
                    nc.vector.memset(ktd[64:128, 0:2].bitcast(F32), 0.0)
                    nc.vector.memset(ktd[64:128, 2 + L:KTDW].bitcast(F32), 0.0)
                bk_col = cols[:, _COL_BK + h:_COL_BK + h + 1]
                for ic in range(NIC):
                    pk = pssm.tile([128, 512], F32, name="ps_small", tag="ps_small")
                    for kb in range(NKB):
                        _mmr(nc, pk[:], wkd[:, 128 * kb:128 * (kb + 1)],
                             kT[:, L * kb + 512 * ic:L * kb + 512 * (ic + 1)],
                             start=(kb == 0), stop=(kb == NKB - 1))
                    nc.vector.tensor_scalar_add(
                        ktd2[0:64, 3 + 512 * ic:3 + 512 * ic + 512],
                        pk[0:64, :], bk_col[0:64, :])
                    if pairs_:
                        nc.vector.tensor_scalar_add(
                            ktd[0:64, 3 + 512 * ic:3 + 512 * ic + 512],
                            pk[0:64, :], bk_col[0:64, :])
                        nc.vector.tensor_scalar_add(
                            ktd[64:128, 2 + 512 * ic:2 + 512 * ic + 512],
                            pk[64:128, :], bk_col[64:128, :])
                return ktd, ktd2

            # ---------- prep phase (transient tiles): q_sT, then v_aug ----------
            with tc.tile_pool(name="prepq", bufs=1) as prep:
                qT = prep.tile([128, NKB * L], F32R, name="qT")
                WqT = prep.tile([128, NKB * D], F32R, name="WqT")
                nc.sync.dma_start(qT[:], qT_d[:])
                nc.sync.dma_start(WqT[:], WqT_d[:])

                # q projection: head pair p -> q_sT[p] (even head rows 0-63)
                for p in range(H // 2):
                    for ic in range(NIC):
                        pq = pssm.tile([128, 512], F32, name="ps_small",
                                       tag="ps_small")
                        for kb in range(NKB):
                            _mmr(nc, pq[:],
                                 WqT[:, D * kb + 128 * p:D * kb + 128 * (p + 1)],
                                 qT[:, L * kb + 512 * ic:L * kb + 512 * (ic + 1)],
                                 start=(kb == 0), stop=(kb == NKB - 1))
                        nc.vector.tensor_scalar_add(
                            q_sT[p][:, 512 * ic:512 * (ic + 1)], pq[:],
                            cols[:, _COL_BQ + p:_COL_BQ + p + 1])

            kbuf_ready = {0: emit_kproj(0)}

            with tc.tile_pool(name="prepv", bufs=1) as prep:
                vT = prep.tile([128, NKB * L], F32R, name="vT")
                WvT = prep.tile([128, NKB * D], F32R, name="WvT")
                nc.sync.dma_start(vT[:], vT_d[:])
                nc.sync.dma_start(WvT[:], WvT_d[:])

                # v projection -> v_aug (interleaved [64 v | 1 ones] per head)
                for j in range(NIB):
                    pv = pssm.tile([128, D], F32, name="ps_small", tag="ps_small")
                    for kb in range(NKB):
                        _mmr(nc, pv[:],
                             vT[:, L * kb + 128 * j:L * kb + 128 * (j + 1)],
                             WvT[:, D * kb:D * (kb + 1)],
                             start=(kb == 0), stop=(kb == NKB - 1))
                    for h in range(H):
                        nc.vector.tensor_tensor(
                            out=v_aug[j][:, 65 * h:65 * h + 64],
                            in0=pv[:, 64 * h:64 * (h + 1)],
                            in1=bv_bc[:, 64 * h:64 * (h + 1)],
                            op=mybir.AluOpType.add)
                    ones_v = v_aug[j].rearrange("p (h c) -> p h c", c=65)[:, :, 64:65]
                    nc.vector.memset(ones_v.bitcast(F32), 1.0)

            # ---------- head loop (software-pipelined) ----------
            # Head h's i-major (attn-output) blocks are emitted interleaved
            # with head h+1's j-major blocks so PSUM slots and engines
            # alternate between the two phases with no serial join.

            def emit_natural_block(st8, ib):
                hh, stacks_, negln_ = st8[0], st8[1], st8[2]
                if ib % 2 == 0:
                    st8[3] = work.tile([128, 2 * L], F32, name="attn_t",
                                       tag="attn_t", bufs=2)
                attn_t = st8[3]
                for jc in range(NIC):
                    pS = psS.tile([128, 512], F32, name="psS", tag="psS")
                    for bi, (st, kbuf, tA) in enumerate(stacks_):
                        _mmr(nc, pS[:],
                             st[:, 128 * ib:128 * (ib + 1)],
                             kbuf[:, 512 * jc + tA:512 * jc + tA + 512],
                             start=(bi == 0), stop=(bi == len(stacks_) - 1))
                    half = L * (ib % 2) + 512 * jc
                    nc.scalar.activation(attn_t[:, half:half + 512], pS[:],
                                         mybir.ActivationFunctionType.Exp,
                                         scale=inv_temper,
                                         bias=negln_[:, ib:ib + 1])
                if ib % 2 == 1:
                    src = attn_t.rearrange("p (c f) -> p c f", c=2)
                    qtr = ib // 2
                    dst = attn_d[hh, 256 * qtr:256 * (qtr + 1), :].rearrange(
                        "(c p) f -> p c f", p=128)
                    nc.sync.dma_start(dst, src)
                    if hh == 0:
                        dsto = oha_d[256 * qtr:256 * (qtr + 1), :].rearrange(
                            "(c p) f -> p c f", p=128)
                        nc.sync.dma_start(dsto, src)

            pending = None
            for h in range(H):
                p, r = h // 2, 64 * (h % 2)
                pairs, tail = _head_blocks(h)
                qs = q_sT[p]

                # --- k projection (duplicated rows) into shifted key buffers ---
                # kTD rows 0-63:  k_sT[h][c, u] at columns u = j + 3   (k_pad index)
                # kTD rows 64-127: same shifted by one key position
                # kTD2: rows 0-63 as kTD, row 64 = ones (bias lane), rows 65+ = 0
                if h in kbuf_ready:
                    ktd, ktd2 = kbuf_ready.pop(h)
                else:
                    ktd, ktd2 = emit_kproj(h)

                # --- Kt stacks over the stacked contraction axis ---
                # pair pi: rows 0-63 = q_s @ WkerT_tA + bker_tA; rows 64-127 = tap tB
                # tail:    rows 0-63 = tap t_tail; row 64 = bias_tot[i]; rows 65+ = 0
                stacks = []  # (sbuf_tile, key_buffer, top_tap)
                for pi, (tA, tB) in enumerate(pairs):
                    st = stack_pool.tile([128, L], F32R, name=f"stack{pi}",
                                         tag=f"stack{pi}")
                    bcol = cols[:, _COL_BKER + _BKER_IDX[(h, pi)]:
                                _COL_BKER + _BKER_IDX[(h, pi)] + 1]
                    for ic in range(NIC):
                        pkt = pssm.tile([128, 512], F32, name="ps_small",
                                        tag="ps_small")
                        _mmr(nc, pkt[:], Wkp[(h, pi)][:],
                             qs[:, 512 * ic:512 * (ic + 1)],
                             start=True, stop=True)
                        nc.vector.tensor_scalar_add(
                            st[:, 512 * ic:512 * (ic + 1)], pkt[:], bcol)
                    stacks.append((st, ktd, tA))

                st_tail = stack_pool.tile([128, L], F32R, name="stack_tail",
                                          tag="stack_tail")
                bcol_t = cols[:, _COL_BKER + _BKER_IDX[(h, "tail")]:
                              _COL_BKER + _BKER_IDX[(h, "tail")] + 1]
                nc.gpsimd.memset(st_tail[64:128, :].bitcast(F32), 0.0)
                for ic in range(NIC):
                    pkt = pssm.tile([128, 512], F32, name="ps_small", tag="ps_small")
                    _mmr(nc, pkt[0:65, :], Wkt[h][:],
                         qs[:, 512 * ic:512 * (ic + 1)],
                         start=True, stop=True)
                    nc.vector.tensor_scalar_add(
                        st_tail[0:65, 512 * ic:512 * (ic + 1)], pkt[0:65, :],
                        bcol_t[0:65, :])
                stacks.append((st_tail, ktd2, tail))

                if PIPE2 and pending is not None:
                    for ib2 in range(NIB):
                        emit_natural_block(pending, ib2)
                    pending = None

                # --- j-major: S_T[j, i] -> exp -> ctx; interleave prev natural ---
                pctx = psctx.tile([65, L], F32, name="psctx", tag="psctx")
                for jb in range(NIB):
                    expT = work.tile([128, L], F32R, name="expT", tag="expT")
                    for ic in range(NIC):
                        pT = psS.tile([128, 512], F32, name="psS", tag="psS")
                        for bi, (st, kbuf, tA) in enumerate(stacks):
                            _mmr(nc, pT[:],
                                 kbuf[:, 128 * jb + tA:128 * jb + tA + 128],
                                 st[:, 512 * ic:512 * (ic + 1)],
                                 start=(bi == 0), stop=(bi == len(stacks) - 1))
                        nc.scalar.activation(expT[:, 512 * ic:512 * (ic + 1)],
                                             pT[:],
                                             mybir.ActivationFunctionType.Exp,
                                             scale=inv_temper)
                        _mmr(nc, pctx[:, 512 * ic:512 * (ic + 1)],
                             v_aug[jb][:, 65 * h:65 * (h + 1)],
                             expT[:, 512 * ic:512 * (ic + 1)],
                             start=(jb == 0), stop=(jb == NIB - 1))
                    if INTERLEAVE and pending is not None:
                        emit_natural_block(pending, jb)
                if INTERLEAVE:
                    pending = None

                # --- softmax denominators: sums row -> -ln cols + recip bcast ---
                sums_row = rows_pool.tile([1, L], F32, name="sums_row", tag="sums_row")
                nc.vector.tensor_copy(sums_row[:], pctx[64:65, :])
                ln_row = rows_pool.tile([1, L], F32, name="ln_row", tag="ln_row")
                nc.scalar.activation(ln_row[:], sums_row[:],
                                     mybir.ActivationFunctionType.Ln)
                negln = work.tile([128, NIB], F32, name="negln", tag="negln")
                for ib in range(NIB):
                    pcol = pssm.tile([128, 512], F32, name="ps_small",
                                     tag="ps_small")
                    nc.tensor.matmul(pcol[:, 0:1],
                                     ln_row[:, 128 * ib:128 * (ib + 1)],
                                     neg1[:], start=True, stop=True)
                    nc.vector.tensor_copy(negln[:, ib:ib + 1], pcol[:, 0:1])

                rcp_row = rows_pool.tile([1, L], F32, name="rcp_row", tag="rcp_row")
                nc.vector.reciprocal(rcp_row[:], sums_row[:])
                bc_sb = work.tile([64, L], F32, name="bc_sb", tag="bc_sb")
                for ic in range(NIC):
                    pbc = pssm.tile([128, 512], F32, name="ps_small", tag="ps_small")
                    nc.tensor.matmul(pbc[0:64, :], ones_col[:],
                                     rcp_row[:, 512 * ic:512 * (ic + 1)],
                                     start=True, stop=True)
                    nc.vector.tensor_copy(bc_sb[:, 512 * ic:512 * (ic + 1)],
                                          pbc[0:64, :])
                nc.vector.tensor_tensor(
                    out=ctxT[p][r:r + 64, :],
                    in0=pctx[0:64, :], in1=bc_sb[:],
                    op=mybir.AluOpType.mult)

                if INTERLEAVE or PIPE2:
                    pending = [h, stacks, negln, None]
                else:
                    own = [h, stacks, negln, None]
                    for ib in range(NIB):
                        emit_natural_block(own, ib)

            if INTERLEAVE or PIPE2:
                for ib in range(NIB):
                    emit_natural_block(pending, ib)
                pending = None

            # ---------- output projection ----------
            for ib in range(NIB):
                po = pssm.tile([128, 512], F32, name="ps_small", tag="ps_small")
                for kb in range(NKB):
                    _mmr(nc, po[:], ctxT[kb][:, 128 * ib:128 * (ib + 1)],
                         WpT[:, D * kb:D * (kb + 1)],
                         start=(kb == 0), stop=(kb == NKB - 1))
                out_t = work.tile([128, D], F32, name="out_t", tag="out_t")
                nc.vector.tensor_tensor(out=out_t[:], in0=po[:], in1=bp_bc[:],
                                        op=mybir.AluOpType.add)
                nc.sync.dma_start(out_d[128 * ib:128 * (ib + 1), :], out_t[:])

    nc.compile()
    return nc


def _pack_kblocks(x, inner):
    """(D, X) -> (128, NKB * X) with block kb at columns [kb*X, (kb+1)*X)."""
    del inner
    Dd, X = x.shape
    out = np.empty((128, NKB * X), np.float32)
    for kb in range(NKB):
        out[:, X * kb:X * (kb + 1)] = x[128 * kb:128 * (kb + 1), :]
    return out


def _prep_weights(Wq, bq, Wk, bk, Wv, bv, Wker, bker, Wqb, bqb, bias_b,
                  Wproj, bproj):
    w = {}
    w["WqTp"] = _pack_kblocks(np.ascontiguousarray(Wq.T), D)
    WkT = Wk.T
    wkd = np.zeros((H, 128, D), np.float32)
    for h in range(H):
        blk = WkT[:, 64 * h:64 * (h + 1)]           # (D, 64): [kd, c]
        dup = np.concatenate([blk, blk], axis=1)    # (D, 128): [kd, m]
        for kb in range(NKB):
            wkd[h, :, 128 * kb:128 * (kb + 1)] = dup[128 * kb:128 * (kb + 1), :]
    w["Wk_dup"] = wkd
    w["WvTp"] = _pack_kblocks(np.ascontiguousarray(Wv.T), D)
    w["WpTp"] = _pack_kblocks(np.ascontiguousarray(Wproj.T), D)
    wkp = np.zeros((N_PAIRS, 128, 128), np.float32)
    wkt = np.zeros((H, 128, 65), np.float32)
    for h in range(H):
        rr = 64 * (h % 2)
        pairs, tail = _head_blocks(h)
        for pi, (tA, tB) in enumerate(pairs):
            idx = _PAIR_IDX[(h, pi)]
            wkp[idx, rr:rr + 64, 0:64] = Wker[h, :, tA, :].T
            wkp[idx, rr:rr + 64, 64:128] = Wker[h, :, tB, :].T
        wkt[h, rr:rr + 64, 0:64] = Wker[h, :, tail, :].T
        wkt[h, rr:rr + 64, 64] = Wqb[h]
    w["wker_pairs"] = wkp
    w["wker_tails"] = wkt
    cols = np.zeros((128, N_COLS), np.float32)
    for h in range(H):
        p, r = h // 2, 64 * (h % 2)
        cols[r:r + 64, _COL_BQ + p] = bq[64 * h:64 * (h + 1)]
        cols[r:r + 64, _COL_WQB + p] = Wqb[h]
        cols[:, _COL_BK + h] = np.concatenate([bk[64 * h:64 * (h + 1)]] * 2)
        pairs, tail = _head_blocks(h)
        for pi, (tA, tB) in enumerate(pairs):
            cols[0:64, _COL_BKER + _BKER_IDX[(h, pi)]] = bker[h, :, tA]
            cols[64:128, _COL_BKER + _BKER_IDX[(h, pi)]] = bker[h, :, tB]
        cols[0:64, _COL_BKER + _BKER_IDX[(h, "tail")]] = bker[h, :, tail]
        cols[64, _COL_BKER + _BKER_IDX[(h, "tail")]] = bqb[h] + bias_b[h]
    w["cols_pack"] = cols
    w["bv_bc"] = np.ascontiguousarray(np.broadcast_to(bv, (128, D)))
    w["bproj_bc"] = np.ascontiguousarray(np.broadcast_to(bproj, (128, D)))
    cb = [float(bqb[h] + bias_b[h]) for h in range(H)]
    return w, cb


def kernel(q, k, v, attn_mask, Wq, bq, Wk, bk, Wv, bv,
           Wker, bker, Wqb, bqb, bias_b, Wproj, bproj):
    del attn_mask  # all-False per the input spec; where(mask, -inf, .) is a no-op
    arrs = {n: np.asarray(a, np.float32) for n, a in dict(
        q=q, k=k, v=v, Wq=Wq, bq=bq, Wk=Wk, bk=bk, Wv=Wv, bv=bv, Wker=Wker,
        bker=bker, Wqb=Wqb, bqb=bqb, bias_b=bias_b, Wproj=Wproj,
        bproj=bproj).items()}

    w, cb = _prep_weights(
        arrs["Wq"], arrs["bq"], arrs["Wk"], arrs["bk"], arrs["Wv"], arrs["bv"],
        arrs["Wker"], arrs["bker"], arrs["Wqb"], arrs["bqb"], arrs["bias_b"],
        arrs["Wproj"], arrs["bproj"])

    nc = build_program(cb)

    in_maps = []
    for b in range(NCORES):
        m = dict(w)
        m["qTp"] = _pack_kblocks(arrs["q"][b].T, L)
        m["kTp"] = _pack_kblocks(arrs["k"][b].T, L)
        m["vTp"] = _pack_kblocks(arrs["v"][b].T, L)
        in_maps.append(m)

    global LAST_RES
    res = run_bass_kernel_spmd(nc, in_maps, list(range(NCORES)))
    LAST_RES = res
    results = res.results

    out = np.stack([results[b]["out"] for b in range(NCORES)])
    attn = np.stack([results[b]["attn"] for b in range(NCORES)])
    oha = np.stack([results[b]["oha"] for b in range(NCORES)])
    return out, attn, oha


# revision 35
# speedup vs baseline: 1.0147x; 1.0070x over previous
"""Trainium2 Bass kernel for nn_MultiHeadAttention_75754633167392.

Multi-head attention with a dynamic per-query conv1d over keys
(per-head kernel widths KWS = [1,1,1,1,3,3,5,7], zero-padded to 7 taps).

Sharding: pure data-parallel over batch — B == n_cores == 8, one batch
element per NeuronCore, no collectives.

Per-core algorithm (fp32 data, float32r matmuls):
  - host pre-transposes q/k/v to (D, L) so contraction over D feeds the
    PE naturally; all weights are host-packed into lhsT layouts.
  - attention logits S[i,j] = bias_q[i] + bias_b
        + sum_t (q_s @ WkerT_t + bker_t)[i,:] . k_pad[j+t,:]
    are evaluated as matmuls over a stacked contraction axis: pairs of
    adjacent taps (t, t+1) are stacked into 128-row operands, using a
    key buffer kTD whose lower 64 partitions hold k_sT and whose upper
    64 partitions hold k_sT shifted by one key position.  The per-query
    bias rides along as one extra contraction row (lhs row = bias_tot[i],
    rhs row = 1).  Only nonzero taps are computed.
  - S is computed in BOTH orientations on the PE (j-major first for the
    attn @ v contraction, then i-major for the attn output); operands are
    identical, only stationary/moving roles swap.
  - the j-major phase also yields the softmax denominators (ones column
    appended to v); the i-major exp then applies bias = -ln(sums[i]) so
    the attn output comes out of the Activation engine already
    normalized (softmax skips max-subtraction: logit rowmax <= ~15).
  - per-head 64-row operands for even/odd heads live in the lower/upper
    64 partitions of shared tiles; matmuls address them via row/col
    tile_position offsets.
"""

import numpy as np

import concourse.bass as bass
import concourse.bacc as bacc
import concourse.mybir as mybir
import concourse.tile as tile
from concourse.bass_utils import run_bass_kernel_spmd

F32 = mybir.dt.float32
F32R = mybir.dt.float32r
LAST_RES = None
import os
INTERLEAVE = os.environ.get("KERN_INTERLEAVE", "0") == "1"
PIPE2 = os.environ.get("KERN_PIPE2", "1") == "1"

B, L, D, H, DK, KW, PAD = 8, 1024, 512, 8, 64, 7, 3
KWS = [1, 1, 1, 1, 3, 3, 5, 7]
TEMPER = float(DK) ** 0.5
NCORES = 8
NIB = L // 128            # 8 row blocks of 128
NIC = L // 512            # 2 column chunks of 512
NKB = D // 128            # 4 contraction blocks of 128
KTDW = L + 8              # key buffer width (3+1024+5 incl. shift slack)


def _head_taps(h):
    kw = KWS[h]
    off = (KW - kw) // 2
    return list(range(off, off + kw))


def _head_blocks(h):
    """Split the head's nonzero taps into adjacent pairs + one tail tap."""
    taps = _head_taps(h)
    pairs = []
    while len(taps) > 1:
        pairs.append((taps[0], taps[1]))
        taps = taps[2:]
    return pairs, taps[0]


# Wker matmul tiles: per (head, pair) a (128, 128) lhsT with tap A weights in
# columns 0-63 and tap B in 64-127, rows 64*(h%2)..+64 (zeros elsewhere so the
# full-128 contraction with the shared q_sT tile nulls the sibling head).
# Per head one (128, 65) tail tile: tap columns 0-63 plus Wqb bias column 64.
_PAIR_IDX = {}
for _h in range(H):
    _pairs, _ = _head_blocks(_h)
    for _pi in range(len(_pairs)):
        _PAIR_IDX[(_h, _pi)] = len(_PAIR_IDX)
N_PAIRS = len(_PAIR_IDX)  # 7

_BKER_IDX = {}
for _h in range(H):
    _pairs, _tail = _head_blocks(_h)
    for _pi, _ in enumerate(_pairs):
        _BKER_IDX[(_h, _pi)] = len(_BKER_IDX)
    _BKER_IDX[(_h, "tail")] = len(_BKER_IDX)
N_BKER = len(_BKER_IDX)   # 15

# cols_pack column layout: bq_pair (4) | Wqb_pair (4) | bk_dup (8) | bker (15)
_COL_BQ = 0
_COL_WQB = 4
_COL_BK = 8
_COL_BKER = 16
N_COLS = 16 + N_BKER


def _mmr(nc, out, lhsT, rhs, **kw):
    nc.tensor.matmul(out, lhsT, rhs, **kw)


def build_program(cb):
    """Build the single-core Bass program. cb[h] = bqb[h] + bias_b[h]."""
    nc = bacc.Bacc(None, target_bir_lowering=False)

    qT_d = nc.dram_tensor("qTp", [128, NKB * L], F32R, kind="ExternalInput")
    kT_d = nc.dram_tensor("kTp", [128, NKB * L], F32R, kind="ExternalInput")
    vT_d = nc.dram_tensor("vTp", [128, NKB * L], F32R, kind="ExternalInput")
    WqT_d = nc.dram_tensor("WqTp", [128, NKB * D], F32R, kind="ExternalInput")
    WvT_d = nc.dram_tensor("WvTp", [128, NKB * D], F32R, kind="ExternalInput")
    WpT_d = nc.dram_tensor("WpTp", [128, NKB * D], F32R, kind="ExternalInput")
    Wkd_d = nc.dram_tensor("Wk_dup", [H, 128, D], F32R, kind="ExternalInput")
    Wkp_d = nc.dram_tensor("wker_pairs", [N_PAIRS, 128, 128], F32R, kind="ExternalInput")
    Wkt_d = nc.dram_tensor("wker_tails", [H, 128, 65], F32R, kind="ExternalInput")
    cols_d = nc.dram_tensor("cols_pack", [128, N_COLS], F32, kind="ExternalInput")
    bv_d = nc.dram_tensor("bv_bc", [128, D], F32, kind="ExternalInput")
    bp_d = nc.dram_tensor("bproj_bc", [128, D], F32, kind="ExternalInput")

    out_d = nc.dram_tensor("out", [L, D], F32, kind="ExternalOutput")
    attn_d = nc.dram_tensor("attn", [H, L, L], F32, kind="ExternalOutput")
    oha_d = nc.dram_tensor("oha", [L, L], F32, kind="ExternalOutput")

    inv_temper = 1.0 / TEMPER

    with tile.TileContext(nc) as tc:
        with (
            tc.tile_pool(name="persist", bufs=1) as pp,
            tc.tile_pool(name="ktd", bufs=2) as ktd_pool,
            tc.tile_pool(name="wkd", bufs=2) as wkd_pool,
            tc.tile_pool(name="stack", bufs=2) as stack_pool,
            tc.tile_pool(name="work", bufs=2) as work,
            tc.tile_pool(name="rows", bufs=1) as rows_pool,
            tc.tile_pool(name="psS", bufs=int(os.environ.get("PSS_BUFS", "4")), space="PSUM") as psS,
            tc.tile_pool(name="psctx", bufs=int(os.environ.get("PSCTX_BUFS", "1")), space="PSUM") as psctx,
            tc.tile_pool(name="pssmall", bufs=int(os.environ.get("PSSM_BUFS", "2")), space="PSUM") as pssm,
        ):
            # ---------- persistent operands ----------
            kT = pp.tile([128, NKB * L], F32R, name="kT")
            nc.sync.dma_start(kT[:], kT_d[:])
            Wkp = {}
            for (hh, pi), idx in _PAIR_IDX.items():
                w = pp.tile([128, 128], F32R, name=f"Wkp{idx}")
                nc.sync.dma_start(w[:], Wkp_d[idx, :, :])
                Wkp[(hh, pi)] = w
            Wkt = []
            for hh in range(H):
                w = pp.tile([128, 65], F32R, name=f"Wkt{hh}")
                nc.sync.dma_start(w[:], Wkt_d[hh, :, :])
                Wkt.append(w)
            cols = pp.tile([128, N_COLS], F32, name="cols")
            nc.sync.dma_start(cols[:], cols_d[:])
            bv_bc = pp.tile([128, D], F32, name="bv_bc")
            nc.sync.dma_start(bv_bc[:], bv_d[:])
            bp_bc = pp.tile([128, D], F32, name="bp_bc")
            nc.sync.dma_start(bp_bc[:], bp_d[:])
            WpT = pp.tile([128, NKB * D], F32R, name="WpT")
            nc.sync.dma_start(WpT[:], WpT_d[:])
            ones_col = pp.tile([1, 64], F32, name="ones_col")
            nc.vector.memset(ones_col[:], 1.0)
            neg1 = pp.tile([1, 1], F32, name="neg1")
            nc.vector.memset(neg1[:], -1.0)

            v_aug = [pp.tile([128, 65 * H], F32R, name=f"vaug{j}") for j in range(NIB)]
            q_sT = [pp.tile([128, L], F32R, name=f"qsT{p}") for p in range(H // 2)]
            ctxT = [pp.tile([128, L], F32R, name=f"ctxT{i}") for i in range(NKB)]

            # ---------- k-projection / key-buffer builder ----------
            def emit_kproj(h):
                pairs_, _tail = _head_blocks(h)
                wkd = wkd_pool.tile([128, D], F32R, name="wkd", tag="wkd")
                nc.sync.dma_start(wkd[:], Wkd_d[h, :, :])
                ktd2 = ktd_pool.tile([128, KTDW], F32R, name="ktd2", tag="ktd2")
                nc.gpsimd.memset(ktd2[64:128, :].bitcast(F32), 0.0)
                nc.gpsimd.memset(ktd2[64:65, :].bitcast(F32), 1.0)
                nc.vector.memset(ktd2[0:64, 0:3].bitcast(F32), 0.0)
                nc.vector.memset(ktd2[0:64, 3 + L:KTDW].bitcast(F32), 0.0)
                ktd = None
                if pairs_:
                    ktd = ktd_pool.tile([128, KTDW], F32R, name="ktd", tag="ktd")
                    nc.vector.memset(ktd[0:64, 0:3].bitcast(F32), 0.0)
                    nc.vector.memset(ktd[0:64, 3 + L:KTDW].bitcast(F32), 0.0)
                    nc.vector.memset(ktd[64:128, 0:2].bitcast(F32), 0.0)
                    nc.vector.memset(ktd[64:128, 2 + L:KTDW].bitcast(F32), 0.0)
                bk_col = cols[:, _COL_BK + h:_COL_BK + h + 1]
                for ic in range(NIC):
                    pk = pssm.tile([128, 512], F32, name="ps_small", tag="ps_small")
                    for kb in range(NKB):
                        _mmr(nc, pk[:], wkd[:, 128 * kb:128 * (kb + 1)],
                             kT[:, L * kb + 512 * ic:L * kb + 512 * (ic + 1)],
                             start=(kb == 0), stop=(kb == NKB - 1))
                    nc.vector.tensor_scalar_add(
                        ktd2[0:64, 3 + 512 * ic:3 + 512 * ic + 512],
                        pk[0:64, :], bk_col[0:64, :])
                    if pairs_:
                        nc.vector.tensor_scalar_add(
                            ktd[0:64, 3 + 512 * ic:3 + 512 * ic + 512],
                            pk[0:64, :], bk_col[0:64, :])
                        nc.vector.tensor_scalar_add(
                            ktd[64:128, 2 + 512 * ic:2 + 512 * ic + 512],
                            pk[64:128, :], bk_col[64:128, :])
                return ktd, ktd2

            # ---------- prep phase (transient tiles): q_sT, then v_aug ----------
            with tc.tile_pool(name="prepq", bufs=1) as prep:
                qT = prep.tile([128, NKB * L], F32R, name="qT")
                WqT = prep.tile([128, NKB * D], F32R, name="WqT")
                nc.sync.dma_start(qT[:], qT_d[:])
                nc.sync.dma_start(WqT[:], WqT_d[:])

                # q projection: head pair p -> q_sT[p] (even head rows 0-63)
                for p in range(H // 2):
                    for ic in range(NIC):
                        pq = pssm.tile([128, 512], F32, name="ps_small",
                                       tag="ps_small")
                        for kb in range(NKB):
                            _mmr(nc, pq[:],
                                 WqT[:, D * kb + 128 * p:D * kb + 128 * (p + 1)],
                                 qT[:, L * kb + 512 * ic:L * kb + 512 * (ic + 1)],
                                 start=(kb == 0), stop=(kb == NKB - 1))
                        nc.vector.tensor_scalar_add(
                            q_sT[p][:, 512 * ic:512 * (ic + 1)], pq[:],
                            cols[:, _COL_BQ + p:_COL_BQ + p + 1])

            kbuf_ready = {0: emit_kproj(0)}

            with tc.tile_pool(name="prepv", bufs=1) as prep:
                vT = prep.tile([128, NKB * L], F32R, name="vT")
                WvT = prep.tile([128, NKB * D], F32R, name="WvT")
                nc.sync.dma_start(vT[:], vT_d[:])
                nc.sync.dma_start(WvT[:], WvT_d[:])

                # v projection -> v_aug (interleaved [64 v | 1 ones] per head)
                for j in range(NIB):
                    pv = pssm.tile([128, D], F32, name="ps_small", tag="ps_small")
                    for kb in range(NKB):
                        _mmr(nc, pv[:],
                             vT[:, L * kb + 128 * j:L * kb + 128 * (j + 1)],
                             WvT[:, D * kb:D * (kb + 1)],
                             start=(kb == 0), stop=(kb == NKB - 1))
                    for h in range(H):
                        nc.vector.tensor_tensor(
                            out=v_aug[j][:, 65 * h:65 * h + 64],
                            in0=pv[:, 64 * h:64 * (h + 1)],
                            in1=bv_bc[:, 64 * h:64 * (h + 1)],
                            op=mybir.AluOpType.add)
                    ones_v = v_aug[j].rearrange("p (h c) -> p h c", c=65)[:, :, 64:65]
                    nc.vector.memset(ones_v.bitcast(F32), 1.0)

            # ---------- head loop (software-pipelined) ----------
            # Head h's i-major (attn-output) blocks are emitted interleaved
            # with head h+1's j-major blocks so PSUM slots and engines
            # alternate between the two phases with no serial join.

            def emit_natural_block(st8, ib):
                hh, stacks_, negln_ = st8[0], st8[1], st8[2]
                if ib % 2 == 0:
                    st8[3] = work.tile([128, 2 * L], F32, name="attn_t",
                                       tag="attn_t", bufs=2)
                attn_t = st8[3]
                for jc in range(NIC):
                    pS = psS.tile([128, 512], F32, name="psS", tag="psS")
                    for bi, (st, kbuf, tA) in enumerate(stacks_):
                        _mmr(nc, pS[:],
                             st[:, 128 * ib:128 * (ib + 1)],
                             kbuf[:, 512 * jc + tA:512 * jc + tA + 512],
                             start=(bi == 0), stop=(bi == len(stacks_) - 1))
                    half = L * (ib % 2) + 512 * jc
                    nc.scalar.activation(attn_t[:, half:half + 512], pS[:],
                                         mybir.ActivationFunctionType.Exp,
                                         scale=inv_temper,
                                         bias=negln_[:, ib:ib + 1])
                if ib % 2 == 1:
                    src = attn_t.rearrange("p (c f) -> p c f", c=2)
                    qtr = ib // 2
                    dst = attn_d[hh, 256 * qtr:256 * (qtr + 1), :].rearrange(
                        "(c p) f -> p c f", p=128)
                    nc.sync.dma_start(dst, src)
                    if hh == 0:
                        dsto = oha_d[256 * qtr:256 * (qtr + 1), :].rearrange(
                            "(c p) f -> p c f", p=128)
                        nc.sync.dma_start(dsto, src)

            pending = None
            for h in range(H):
                p, r = h // 2, 64 * (h % 2)
                pairs, tail = _head_blocks(h)
                qs = q_sT[p]

                # --- k projection (duplicated rows) into shifted key buffers ---
                # kTD rows 0-63:  k_sT[h][c, u] at columns u = j + 3   (k_pad index)
                # kTD rows 64-127: same shifted by one key position
                # kTD2: rows 0-63 as kTD, row 64 = ones (bias lane), rows 65+ = 0
                if h in kbuf_ready:
                    ktd, ktd2 = kbuf_ready.pop(h)
                else:
                    ktd, ktd2 = emit_kproj(h)

                # --- Kt stacks over the stacked contraction axis ---
                # pair pi: rows 0-63 = q_s @ WkerT_tA + bker_tA; rows 64-127 = tap tB
                # tail:    rows 0-63 = tap t_tail; row 64 = bias_tot[i]; rows 65+ = 0
                stacks = []  # (sbuf_tile, key_buffer, top_tap)
                for pi, (tA, tB) in enumerate(pairs):
                    st = stack_pool.tile([128, L], F32R, name=f"stack{pi}",
                                         tag=f"stack{pi}")
                    bcol = cols[:, _COL_BKER + _BKER_IDX[(h, pi)]:
                                _COL_BKER + _BKER_IDX[(h, pi)] + 1]
                    for ic in range(NIC):
                        pkt = pssm.tile([128, 512], F32, name="ps_small",
                                        tag="ps_small")
                        _mmr(nc, pkt[:], Wkp[(h, pi)][:],
                             qs[:, 512 * ic:512 * (ic + 1)],
                             start=True, stop=True)
                        nc.vector.tensor_scalar_add(
                            st[:, 512 * ic:512 * (ic + 1)], pkt[:], bcol)
                    stacks.append((st, ktd, tA))

                st_tail = stack_pool.tile([128, L], F32R, name="stack_tail",
                                          tag="stack_tail")
                bcol_t = cols[:, _COL_BKER + _BKER_IDX[(h, "tail")]:
                              _COL_BKER + _BKER_IDX[(h, "tail")] + 1]
                nc.gpsimd.memset(st_tail[64:128, :].bitcast(F32), 0.0)
                for ic in range(NIC):
                    pkt = pssm.tile([128, 512], F32, name="ps_small", tag="ps_small")
                    _mmr(nc, pkt[0:65, :], Wkt[h][:],
                         qs[:, 512 * ic:512 * (ic + 1)],
                         start=True, stop=True)
                    nc.vector.tensor_scalar_add(
                        st_tail[0:65, 512 * ic:512 * (ic + 1)], pkt[0:65, :],
                        bcol_t[0:65, :])
                stacks.append((st_tail, ktd2, tail))

                if PIPE2 and pending is not None:
                    for ib2 in range(NIB):
                        emit_natural_block(pending, ib2)
                    pending = None

                if h + 1 < H and (h + 1) not in kbuf_ready:
                    kbuf_ready[h + 1] = emit_kproj(h + 1)

                # --- j-major: S_T[j, i] -> exp -> ctx; interleave prev natural ---
                pctx = psctx.tile([65, L], F32, name="psctx", tag="psctx")
                for jb in range(NIB):
                    expT = work.tile([128, L], F32R, name="expT", tag="expT")
                    for ic in range(NIC):
                        pT = psS.tile([128, 512], F32, name="psS", tag="psS")
                        for bi, (st, kbuf, tA) in enumerate(stacks):
                            _mmr(nc, pT[:],
                                 kbuf[:, 128 * jb + tA:128 * jb + tA + 128],
                                 st[:, 512 * ic:512 * (ic + 1)],
                                 start=(bi == 0), stop=(bi == len(stacks) - 1))
                        nc.scalar.activation(expT[:, 512 * ic:512 * (ic + 1)],
                                             pT[:],
                                             mybir.ActivationFunctionType.Exp,
                                             scale=inv_temper)
                        _mmr(nc, pctx[:, 512 * ic:512 * (ic + 1)],
                             v_aug[jb][:, 65 * h:65 * (h + 1)],
                             expT[:, 512 * ic:512 * (ic + 1)],
                             start=(jb == 0), stop=(jb == NIB - 1))
                    if INTERLEAVE and pending is not None:
                        emit_natural_block(pending, jb)
                if INTERLEAVE:
                    pending = None

                # --- softmax denominators: sums row -> -ln cols + recip bcast ---
                sums_row = rows_pool.tile([1, L], F32, name="sums_row", tag="sums_row")
                nc.vector.tensor_copy(sums_row[:], pctx[64:65, :])
                ln_row = rows_pool.tile([1, L], F32, name="ln_row", tag="ln_row")
                nc.scalar.activation(ln_row[:], sums_row[:],
                                     mybir.ActivationFunctionType.Ln)
                negln = work.tile([128, NIB], F32, name="negln", tag="negln")
                for ib in range(NIB):
                    pcol = pssm.tile([128, 512], F32, name="ps_small",
                                     tag="ps_small")
                    nc.tensor.matmul(pcol[:, 0:1],
                                     ln_row[:, 128 * ib:128 * (ib + 1)],
                                     neg1[:], start=True, stop=True)
                    nc.vector.tensor_copy(negln[:, ib:ib + 1], pcol[:, 0:1])

                rcp_row = rows_pool.tile([1, L], F32, name="rcp_row", tag="rcp_row")
                nc.vector.reciprocal(rcp_row[:], sums_row[:])
                bc_sb = work.tile([64, L], F32, name="bc_sb", tag="bc_sb")
                for ic in range(NIC):
                    pbc = pssm.tile([128, 512], F32, name="ps_small", tag="ps_small")
                    nc.tensor.matmul(pbc[0:64, :], ones_col[:],
                                     rcp_row[:, 512 * ic:512 * (ic + 1)],
                                     start=True, stop=True)
                    nc.vector.tensor_copy(bc_sb[:, 512 * ic:512 * (ic + 1)],
                                          pbc[0:64, :])
                nc.vector.tensor_tensor(
                    out=ctxT[p][r:r + 64, :],
                    in0=pctx[0:64, :], in1=bc_sb[:],
                    op=mybir.AluOpType.mult)

                if INTERLEAVE or PIPE2:
                    pending = [h, stacks, negln, None]
                else:
                    own = [h, stacks, negln, None]
                    for ib in range(NIB):
                        emit_natural_block(own, ib)

            if INTERLEAVE or PIPE2:
                for ib in range(NIB):
                    emit_natural_block(pending, ib)
                pending = None

            # ---------- output projection ----------
            for ib in range(NIB):
                po = pssm.tile([128, 512], F32, name="ps_small", tag="ps_small")
                for kb in range(NKB):
                    _mmr(nc, po[:], ctxT[kb][:, 128 * ib:128 * (ib + 1)],
                         WpT[:, D * kb:D * (kb + 1)],
                         start=(kb == 0), stop=(kb == NKB - 1))
                out_t = work.tile([128, D], F32, name="out_t", tag="out_t")
                nc.vector.tensor_tensor(out=out_t[:], in0=po[:], in1=bp_bc[:],
                                        op=mybir.AluOpType.add)
                nc.sync.dma_start(out_d[128 * ib:128 * (ib + 1), :], out_t[:])

    nc.compile()
    return nc


def _pack_kblocks(x, inner):
    """(D, X) -> (128, NKB * X) with block kb at columns [kb*X, (kb+1)*X)."""
    del inner
    Dd, X = x.shape
    out = np.empty((128, NKB * X), np.float32)
    for kb in range(NKB):
        out[:, X * kb:X * (kb + 1)] = x[128 * kb:128 * (kb + 1), :]
    return out


def _prep_weights(Wq, bq, Wk, bk, Wv, bv, Wker, bker, Wqb, bqb, bias_b,
                  Wproj, bproj):
    w = {}
    w["WqTp"] = _pack_kblocks(np.ascontiguousarray(Wq.T), D)
    WkT = Wk.T
    wkd = np.zeros((H, 128, D), np.float32)
    for h in range(H):
        blk = WkT[:, 64 * h:64 * (h + 1)]           # (D, 64): [kd, c]
        dup = np.concatenate([blk, blk], axis=1)    # (D, 128): [kd, m]
        for kb in range(NKB):
            wkd[h, :, 128 * kb:128 * (kb + 1)] = dup[128 * kb:128 * (kb + 1), :]
    w["Wk_dup"] = wkd
    w["WvTp"] = _pack_kblocks(np.ascontiguousarray(Wv.T), D)
    w["WpTp"] = _pack_kblocks(np.ascontiguousarray(Wproj.T), D)
    wkp = np.zeros((N_PAIRS, 128, 128), np.float32)
    wkt = np.zeros((H, 128, 65), np.float32)
    for h in range(H):
        rr = 64 * (h % 2)
        pairs, tail = _head_blocks(h)
        for pi, (tA, tB) in enumerate(pairs):
            idx = _PAIR_IDX[(h, pi)]
            wkp[idx, rr:rr + 64, 0:64] = Wker[h, :, tA, :].T
            wkp[idx, rr:rr + 64, 64:128] = Wker[h, :, tB, :].T
        wkt[h, rr:rr + 64, 0:64] = Wker[h, :, tail, :].T
        wkt[h, rr:rr + 64, 64] = Wqb[h]
    w["wker_pairs"] = wkp
    w["wker_tails"] = wkt
    cols = np.zeros((128, N_COLS), np.float32)
    for h in range(H):
        p, r = h // 2, 64 * (h % 2)
        cols[r:r + 64, _COL_BQ + p] = bq[64 * h:64 * (h + 1)]
        cols[r:r + 64, _COL_WQB + p] = Wqb[h]
        cols[:, _COL_BK + h] = np.concatenate([bk[64 * h:64 * (h + 1)]] * 2)
        pairs, tail = _head_blocks(h)
        for pi, (tA, tB) in enumerate(pairs):
            cols[0:64, _COL_BKER + _BKER_IDX[(h, pi)]] = bker[h, :, tA]
            cols[64:128, _COL_BKER + _BKER_IDX[(h, pi)]] = bker[h, :, tB]
        cols[0:64, _COL_BKER + _BKER_IDX[(h, "tail")]] = bker[h, :, tail]
        cols[64, _COL_BKER + _BKER_IDX[(h, "tail")]] = bqb[h] + bias_b[h]
    w["cols_pack"] = cols
    w["bv_bc"] = np.ascontiguousarray(np.broadcast_to(bv, (128, D)))
    w["bproj_bc"] = np.ascontiguousarray(np.broadcast_to(bproj, (128, D)))
    cb = [float(bqb[h] + bias_b[h]) for h in range(H)]
    return w, cb


def kernel(q, k, v, attn_mask, Wq, bq, Wk, bk, Wv, bv,
           Wker, bker, Wqb, bqb, bias_b, Wproj, bproj):
    del attn_mask  # all-False per the input spec; where(mask, -inf, .) is a no-op
    arrs = {n: np.asarray(a, np.float32) for n, a in dict(
        q=q, k=k, v=v, Wq=Wq, bq=bq, Wk=Wk, bk=bk, Wv=Wv, bv=bv, Wker=Wker,
        bker=bker, Wqb=Wqb, bqb=bqb, bias_b=bias_b, Wproj=Wproj,
        bproj=bproj).items()}

    w, cb = _prep_weights(
        arrs["Wq"], arrs["bq"], arrs["Wk"], arrs["bk"], arrs["Wv"], arrs["bv"],
        arrs["Wker"], arrs["bker"], arrs["Wqb"], arrs["bqb"], arrs["bias_b"],
        arrs["Wproj"], arrs["bproj"])

    nc = build_program(cb)

    in_maps = []
    for b in range(NCORES):
        m = dict(w)
        m["qTp"] = _pack_kblocks(arrs["q"][b].T, L)
        m["kTp"] = _pack_kblocks(arrs["k"][b].T, L)
        m["vTp"] = _pack_kblocks(arrs["v"][b].T, L)
        in_maps.append(m)

    global LAST_RES
    res = run_bass_kernel_spmd(nc, in_maps, list(range(NCORES)))
    LAST_RES = res
    results = res.results

    out = np.stack([results[b]["out"] for b in range(NCORES)])
    attn = np.stack([results[b]["attn"] for b in range(NCORES)])
    oha = np.stack([results[b]["oha"] for b in range(NCORES)])
    return out, attn, oha
